# revision 24
# baseline (speedup 1.0000x reference)
"""Trainium2 Bass kernel for neighbor-sum aggregation (GNN message passing).

reference:  out[b, :] = sum_k embed_matrix[neigh_idx[b, k], :]   (B=50000, K=16,
            U=100000, D=512)

Strategy (v5):
- Data-parallel over B across 8 NeuronCores; embed replicated, converted to
  bf16 on the host (harness tolerance 2e-2; this path lands ~4e-3).
- The gather uses batched SWDGE ``dma_gather`` (ONE op per (tile, bank) with
  hundreds of descriptors) instead of per-row ``indirect_dma_start`` ops:
  SWDGE descriptor generation costs ~1us of GpSimd engine time PER OP, which
  made the 784-op-per-core variant GpSimd-bound at ~1.1ms.
- ``dma_gather`` indices are int16 (< 32768), so embed is split into 4 row
  banks of 25000 (+1 zeros row each).  Position i of a gather lands at dest
  [i%128, i//128]; column p of each bank op carries the bank-b neighbors of
  the batch row assigned to partition p, padded with the bank's zeros row.
- Batch rows are profile-sorted on the host (by per-bank neighbor counts) so
  the 128 rows sharing a tile have near-identical bank profiles, keeping the
  zero-row padding (sum_b C_b vs 16) small.  Chunk counts C[t][b] are static
  in the NEFF (shared by all 8 cores = max over cores); the kernel is
  therefore compiled per input set.  The row sort is undone on the host.
- Per tile: <=4 dma_gathers -> in-place bf16 pairwise fold per bank ->
  combine banks into a f32 row -> HWDGE store.
"""

import numpy as np
import ml_dtypes

import concourse.bacc as bacc
import concourse.mybir as mybir
import concourse.tile as tile
from concourse import library_config
from concourse.bass_utils import run_bass_kernel_spmd

N_CORES = 8
B, K = 50000, 16
U, D = 100000, 512
P = 128
TILES = 49                      # output tiles per core
B_SHARD = TILES * P             # 6272 padded rows per core
B_PAD = N_CORES * B_SHARD       # 50176

NBANKS = 4
BR = U // NBANKS                # 25000 rows per bank
BROWS = BR + 1                  # + zeros row (local index BR)

GATH_BUFS = 3
ACC_BUFS = 3

_PLAN = {}


def _make_plan(neigh_idx):
    """Host-side sharding plan: profile-sorted row order, static chunk
    counts C[t][b] (shared across cores), per-core int16 index buffers."""
    idx = np.zeros((B_PAD, K), np.int64)
    idx[:B] = np.asarray(neigh_idx).astype(np.int64)
    bank = idx // BR                                 # [B_PAD, K]
    prof = np.stack([(bank == b).sum(1) for b in range(NBANKS)], 1)
    # group rows whose dominant bank and profile match: minimizes
    # sum_b max-over-tile(n_b), i.e. the zero-row padding bloat
    order = np.lexsort(
        (prof[:, 3], prof[:, 2], prof[:, 1], prof[:, 0],
         prof.max(1), prof.argmax(1))
    )

    # global tile g = t*8 + c holds rows order[g*128:(g+1)*128]
    # C[t][b] = max neighbor count in bank b over the 1024 rows of tile-slot t
    rows_by_slot = order.reshape(TILES, N_CORES, P)
    C = np.zeros((TILES, NBANKS), int)
    for t in range(TILES):
        r = rows_by_slot[t].reshape(-1)
        C[t] = prof[r].max(0)

    w_off = np.zeros((TILES, NBANKS), int)
    w = 0
    for t in range(TILES):
        for b in range(NBANKS):
            w_off[t, b] = w
            w += 8 * C[t, b]
    W = w

    idx_bufs = np.full((N_CORES, P, W), BR, np.int16)  # pad = zeros row
    for c in range(N_CORES):
        for t in range(TILES):
            rows_ct = rows_by_slot[t, c]
            bk = bank[rows_ct]                       # [128, K]
            vals_all = idx[rows_ct]                  # [128, K]
            for b in range(NBANKS):
                Cb = C[t, b]
                if Cb == 0:
                    continue
                val = np.full((Cb, P), BR, np.int16)  # [chunk j, partition p]
                for p in range(P):
                    v = vals_all[p][bk[p] == b] - b * BR
                    val[: len(v), p] = v.astype(np.int16)
                # position i=j*128+p -> idxs[(p%16), j*8 + p//16], x8 replicate
                a16 = val.reshape(Cb, 8, 16).transpose(2, 0, 1).reshape(16, 8 * Cb)
                idx_bufs[c, :, w_off[t, b] : w_off[t, b] + 8 * Cb] = np.tile(
                    a16, (8, 1)
                )
    return {"order": order, "C": C, "w_off": w_off, "W": W, "idx_bufs": idx_bufs}


def build_nc(reps=1):
    plan = _PLAN["plan"]
    C, w_off, W = plan["C"], plan["w_off"], plan["W"]
    CM = C.max(0)                                    # per-bank max chunks
    # SBUF/partition budget (~200KB): gather tiles are sum(CM) KB per buf
    gath_bufs = 4 if int(CM.sum()) <= 42 else GATH_BUFS

    # Queue load balancing: desc-gen overlaps across the 4 SWDGE queues, so
    # don't pin bank b to queue b (the dominant-bank row sort makes one bank's
    # gather much bigger for long runs of tiles).  Split the heaviest bank's
    # gather into two half-ops (disjoint chunk ranges of the same tile), then
    # rank all ops by size and rotate rank->queue across tiles so each queue
    # carries a balanced share of the in-flight descriptor load.
    tile_ops = []                   # per tile: list of (b, chunk_off, n, queue)
    for t in range(TILES):
        parts = []
        sizes = [(int(C[t, b]), b) for b in range(NBANKS) if C[t, b] > 0]
        b_heavy = max(sizes)[1] if sizes else -1
        for Cb, b in sizes:
            if b == b_heavy and Cb >= 4:
                h = Cb // 2
                parts += [(b, 0, h), (b, h, Cb - h)]
            else:
                parts.append((b, 0, Cb))
        parts.sort(key=lambda x: -x[2])
        tile_ops.append(
            [(b, off, n, (t + r) % 4) for r, (b, off, n) in enumerate(parts)]
        )

    nc = bacc.Bacc(
        "TRN2", target_bir_lowering=False, debug=False, num_swdge_queues=4
    )
    idxs = nc.dram_tensor("idxs", [P, W], mybir.dt.int16, kind="ExternalInput")
    banks = [
        nc.dram_tensor(f"emb{b}", [BROWS, D], mybir.dt.bfloat16, kind="ExternalInput")
        for b in range(NBANKS)
    ]
    out = nc.dram_tensor("out", [B_SHARD, D], mybir.dt.float32, kind="ExternalOutput")

    with tile.TileContext(nc) as tc:
        nc.gpsimd.load_library(library_config.mlp)
        with (
            tc.tile_pool(name="idxp", bufs=1) as idx_pool,
            tc.tile_pool(name="gath", bufs=gath_bufs) as gpool,
            tc.tile_pool(name="accp", bufs=ACC_BUFS) as apool,
        ):
            idx_all = idx_pool.tile([P, W], mybir.dt.int16)
            nc.sync.dma_start(out=idx_all[:], in_=idxs.ap())

            def fold(g, Cb):
                # in-place pairwise fold of Cb D-chunks down to chunk 0
                while Cb > 1:
                    H = Cb // 2
                    nc.vector.tensor_tensor(
                        out=g[:, 0 : H * D],
                        in0=g[:, 0 : H * D],
                        in1=g[:, (Cb - H) * D : Cb * D],
                        op=mybir.AluOpType.add,
                    )
                    Cb -= H

            def body():
                for t in range(TILES):
                    gtiles = []
                    gt_by_bank = {}
                    for b, off, n, q in tile_ops[t]:
                        if b not in gt_by_bank:
                            g = gpool.tile([P, int(CM[b]) * D],
                                           mybir.dt.bfloat16, tag=f"g{b}")
                            gt_by_bank[b] = g
                            gtiles.append((g, int(C[t, b])))
                        g = gt_by_bank[b]
                        nc.gpsimd.dma_gather(
                            g[:, off * D : (off + n) * D].rearrange(
                                "p (c e) -> p c e", e=D
                            ),
                            banks[b].ap(),
                            idx_all[
                                :,
                                w_off[t, b] + 8 * off : w_off[t, b] + 8 * (off + n),
                            ],
                            128 * n,
                            128 * n,
                            D,
                            single_packet=False,
                            queue_num=q,
                        )
                    for g, Cb in gtiles:
                        fold(g, Cb)
                    # combine banks (bf16) then final add into f32 acc
                    acc = apool.tile([P, D], mybir.dt.float32, tag="a")
                    gs = [g for g, _ in gtiles]
                    while len(gs) > 2:
                        nc.vector.tensor_tensor(
                            out=gs[0][:, 0:D], in0=gs[0][:, 0:D],
                            in1=gs[-1][:, 0:D], op=mybir.AluOpType.add,
                        )
                        gs = gs[:-1]
                    if len(gs) == 2:
                        nc.vector.tensor_tensor(
                            out=acc[:], in0=gs[0][:, 0:D], in1=gs[1][:, 0:D],
                            op=mybir.AluOpType.add,
                        )
                    else:
                        nc.vector.tensor_copy(out=acc[:], in_=gs[0][:, 0:D])
                    nc.sync.dma_start(
                        out=out.ap()[t * P : (t + 1) * P, :], in_=acc[:]
                    )

            if reps == 1:
                body()
            else:
                with tc.For_i(0, reps, 1):
                    body()
    nc.compile()
    return nc


def make_in_maps(neigh_idx, embed_matrix):
    plan = _make_plan(neigh_idx)
    _PLAN["plan"] = plan
    emb = np.asarray(embed_matrix).astype(ml_dtypes.bfloat16)
    emb_banks = []
    for b in range(NBANKS):
        eb = np.zeros((BROWS, D), ml_dtypes.bfloat16)
        eb[:BR] = emb[b * BR : (b + 1) * BR]
        emb_banks.append(eb)
    return [
        {
            "idxs": np.ascontiguousarray(plan["idx_bufs"][c]),
            **{f"emb{b}": emb_banks[b] for b in range(NBANKS)},
        }
        for c in range(N_CORES)
    ]


def kernel(neigh_idx, embed_matrix):
    in_maps = make_in_maps(neigh_idx, embed_matrix)
    nc = build_nc()
    res = run_bass_kernel_spmd(nc, in_maps, list(range(N_CORES))).results
    plan = _PLAN["plan"]
    # device rows are in profile-sorted order: undo on host
    out_sorted = np.stack([res[c]["out"] for c in range(N_CORES)], axis=0)
    # row order[(t*8+c)*128 + p] was computed by core c, tile t, partition p
    full = np.empty((B_PAD, D), np.float32)
    rows_by_slot = plan["order"].reshape(TILES, N_CORES, P)
    for c in range(N_CORES):
        full[rows_by_slot[:, c, :].reshape(-1)] = out_sorted[c]
    return np.ascontiguousarray(full[:B], dtype=np.float32)
